# revision 17
# baseline (speedup 1.0000x reference)
"""Trainium2 kernel for nn_Dense_Q_MulIn1Out_Conv1D.

The reference "quantum conv" circuit is linear in the state vector: three
RY-rotation layers interleaved with a fixed 512x512 orthogonal entangler.
The whole circuit therefore collapses to one matrix M (512x512), and since
the encoded state has only its first 128 amplitudes nonzero, the <Z> readout
reduces to a quadratic form with a fixed symmetric 128x128 matrix A:

    out[n] = (v_n^T A v_n) / (||v_n||^2 + 1e-12)

where v_n is the (unnormalized) im2col patch of x (C=16 channels x K=8 taps,
channel-major).  A = Md^T Z Md with Md = M[:, :128], Z = diag(+1 x256, -1 x256).

Host side: build A (float64) from entangle_matrix/theta, permute it to
k-major patch order so the on-device im2col is 8 shifted row-block copies.
Device side (per core, 2 of 16 batches): build V [128, 4096] per batch by
DMA, Y = A @ V on TensorE (fp32r), P1 = V*Y, P2 = V*V elementwise, then
ones-vector matmuls reduce partitions to num/den rows of a [16, 512] PSUM
tile; final reciprocal-multiply and one 32KB store.
"""

import numpy as np

_DIM = 512
_D = 128
_K = 8
_C = 16
_NQ = 9
_B = 16
_L = 4096
_L_OUT = _L - _K + 1  # 4089
_N_CORES = 8
_B_PER_CORE = _B // _N_CORES  # 2
_NCHUNK = 8  # 512-column chunks per batch
_CHUNK = 512

# k-major patch permutation: new index p = k*16 + c  <->  old index c*8 + k
_PERM = np.array([(p % _C) * _K + (p // _C) for p in range(_D)])


def _apply_ry_layer(psi, angles):
    # psi [N, DIM] float64; matches reference._apply_ry_layer
    for q in range(_NQ):
        half = angles[q] * 0.5
        c, s = np.cos(half), np.sin(half)
        left = 2 ** q
        p = psi.reshape(-1, left, 2, _DIM // (2 ** (q + 1)))
        a, b = p[:, :, 0, :].copy(), p[:, :, 1, :].copy()
        psi = np.stack([c * a - s * b, s * a + c * b], axis=2).reshape(-1, _DIM)
    return psi


def _build_amat(entangle_matrix, theta):
    """Collapse the circuit to the k-major-permuted 128x128 quadratic form."""
    U = np.asarray(entangle_matrix, dtype=np.float64)
    th = np.asarray(theta, dtype=np.float64)
    psi = np.eye(_DIM, dtype=np.float64)
    for l in range(th.shape[0]):
        psi = _apply_ry_layer(psi, th[l])
        psi = psi @ U.T
    M = psi.T  # state map: s -> M s
    z = np.concatenate([np.ones(_DIM // 2), -np.ones(_DIM // 2)])
    Md = M[:, :_D]
    A = Md.T @ (z[:, None] * Md)
    A_km = A[np.ix_(_PERM, _PERM)]
    return np.ascontiguousarray(A_km, dtype=np.float32)


_NC_CACHE = {}


def _build_nc():
    import concourse.tile as tile
    from concourse import bacc, mybir

    F32 = mybir.dt.float32
    F32R = mybir.dt.float32r
    AF = mybir.ActivationFunctionType

    nc = bacc.Bacc(
        "TRN2",
        target_bir_lowering=False,
        debug=False,
        num_devices=_N_CORES,
    )
    ngl = _B_PER_CORE * _NCHUNK  # 16 global chunks
    # flat x + 8 pad elements so the im2col window never reads out of bounds
    x = nc.dram_tensor(
        "x", [_B_PER_CORE * _C * _L + _K], F32, kind="ExternalInput"
    ).ap()
    # consts = [A_km | SEL]: A_km [128,128]; SEL [128, 16*ngl] selector blocks
    consts = nc.dram_tensor(
        "consts", [_D, _D + 16 * ngl], F32, kind="ExternalInput"
    ).ap()
    out = nc.dram_tensor(
        "out", [_B_PER_CORE * _NCHUNK, _CHUNK], F32, kind="ExternalOutput"
    ).ap()

    with tile.TileContext(nc) as tc:
        from contextlib import ExitStack

        with ExitStack() as ctx:
            const_pool = ctx.enter_context(tc.tile_pool(name="const", bufs=1))
            v_pool = ctx.enter_context(tc.tile_pool(name="v", bufs=2))
            p_pool = ctx.enter_context(tc.tile_pool(name="p", bufs=3))
            y_pool = ctx.enter_context(tc.tile_pool(name="y", bufs=2, space="PSUM"))
            red_pool = ctx.enter_context(tc.tile_pool(name="red", bufs=1, space="PSUM"))
            o_pool = ctx.enter_context(tc.tile_pool(name="o", bufs=1))

            # One DMA for all constants: A (lhsT of main matmuls) and the
            # selector blocks (sel block g = ones in within-block column g:
            # matmul with lhsT=block_g sums partitions into output row g).
            c_sb = const_pool.tile([_D, _D + 16 * ngl], F32)
            nc.sync.dma_start(c_sb[:].bitcast(F32R), consts[:].bitcast(F32R))
            a_sb = c_sb[:, :_D]
            sel = c_sb[:, _D:]
            # Collapse the const deps into one sync point so downstream
            # instructions don't exceed the per-instruction wait limit.
            tc.strict_bb_all_engine_barrier()

            psum_num = red_pool.tile([16, _CHUNK], F32)
            psum_den = red_pool.tile([16, _CHUNK], F32)

            from bass_rust import AP as RawAP

            # V free size is _L+1 so its partition pitch (4097) can't be
            # coalesced with the 4096-element column runs by the DMA AP
            # balancer (a flat run crossing SBUF partitions is invalid).
            _LV = _L + 1
            for b in range(_B_PER_CORE):
                v = v_pool.tile([_D, _LV], F32)
                # one im2col DMA: dst partition (k*16+c), col n <- x[b, c, n+k].
                # Columns >= L_OUT pick up neighboring-channel values (finite
                # garbage); the host discards them.
                dst = v[:, 0:_L].bitcast(F32R)
                src = RawAP(
                    tensor=x.tensor, offset=b * _C * _L,
                    ap=[[1, _K], [_L, _C], [1, _L]],
                ).bitcast(F32R)
                nc.sync.dma_start(dst, src)
                for j in range(_NCHUNK):
                    vch = v[:, j * _CHUNK : (j + 1) * _CHUNK]
                    py = y_pool.tile([_D, _CHUNK], F32)
                    nc.tensor.matmul(
                        py[:], a_sb.bitcast(F32R), vch.bitcast(F32R),
                        start=True, stop=True,
                    )
                    p1 = p_pool.tile([_D, _CHUNK], F32, tag="p1")
                    nc.vector.tensor_mul(p1[:].bitcast(F32R), vch, py[:])
                    p2 = p_pool.tile([_D, _CHUNK], F32, tag="p2")
                    nc.scalar.activation(p2[:].bitcast(F32R), vch, AF.Square)
                    g = b * _NCHUNK + j
                    selg = sel[:, g * 16 : (g + 1) * 16].bitcast(F32R)
                    nc.tensor.matmul(
                        psum_num[:], selg, p1[:].bitcast(F32R),
                        start=(g == 0), stop=(g == ngl - 1),
                        skip_group_check=True,
                    )
                    nc.tensor.matmul(
                        psum_den[:], selg, p2[:].bitcast(F32R),
                        start=(g == 0), stop=(g == ngl - 1),
                        skip_group_check=True,
                    )

            den_sb = o_pool.tile([16, _CHUNK], F32, tag="den")
            nc.scalar.activation(den_sb[:], psum_den[:], AF.Copy, bias=1e-12)
            rden = o_pool.tile([16, _CHUNK], F32, tag="rden")
            nc.vector.reciprocal_approx_fast(rden[:], den_sb[:])
            out_sb = o_pool.tile([16, _CHUNK], F32, tag="outsb")
            nc.vector.tensor_mul(out_sb[:], psum_num[:], rden[:])
            nc.sync.dma_start(out[:], out_sb[:])

    nc.compile()
    return nc


def get_nc():
    if "nc" not in _NC_CACHE:
        _NC_CACHE["nc"] = _build_nc()
    return _NC_CACHE["nc"]


def kernel(x, entangle_matrix, theta, _trace=False, **trace_kwargs):
    from concourse.bass_utils import run_bass_kernel_spmd

    x = np.asarray(x, dtype=np.float32)
    amat = _build_amat(entangle_matrix, theta)
    ngl = _B_PER_CORE * _NCHUNK
    sel = np.zeros((_D, 16 * ngl), dtype=np.float32)
    for g in range(ngl):
        sel[:, g * 17] = 1.0
    consts = np.ascontiguousarray(np.concatenate([amat, sel], axis=1))

    nc = get_nc()
    pad = np.zeros(_K, dtype=np.float32)
    in_maps = [
        {
            "x": np.concatenate(
                [x[i * _B_PER_CORE : (i + 1) * _B_PER_CORE].reshape(-1), pad]
            ),
            "consts": consts,
        }
        for i in range(_N_CORES)
    ]
    res = run_bass_kernel_spmd(
        nc, in_maps, list(range(_N_CORES)), trace=_trace, **trace_kwargs
    )
    outs = []
    for i in range(_N_CORES):
        o = np.asarray(res.results[i]["out"], dtype=np.float32)
        outs.append(o.reshape(_B_PER_CORE, _NCHUNK * _CHUNK)[:, :_L_OUT])
    full = np.concatenate(outs, axis=0).reshape(_B, 1, 1, _L_OUT)
    if _trace:
        kernel._last_results = res
    return full


# revision 20
# speedup vs baseline: 1.4320x; 1.4320x over previous
"""Trainium2 kernel for nn_Dense_Q_MulIn1Out_Conv1D.

The reference "quantum conv" circuit is linear in the state vector: three
RY-rotation layers interleaved with a fixed 512x512 orthogonal entangler.
The whole circuit therefore collapses to one matrix M (512x512), and since
the encoded state has only its first 128 amplitudes nonzero, the <Z> readout
reduces to a quadratic form with a fixed symmetric 128x128 matrix A:

    out[n] = (v_n^T A v_n) / (||v_n||^2 + 1e-12)

where v_n is the (unnormalized) im2col patch of x (C=16 channels x K=8 taps,
channel-major).  A = Md^T Z Md with Md = M[:, :128], Z = diag(+1 x256, -1 x256).

Host side: build A (float64) from entangle_matrix/theta, permute it to
k-major patch order so the on-device im2col is 8 shifted row-block copies.
Device side (per core, 2 of 16 batches): build V [128, 4096] per batch by
DMA, Y = A @ V on TensorE (fp32r), P1 = V*Y, P2 = V*V elementwise, then
ones-vector matmuls reduce partitions to num/den rows of a [16, 512] PSUM
tile; final reciprocal-multiply and one 32KB store.
"""

import numpy as np

_DIM = 512
_D = 128
_K = 8
_C = 16
_NQ = 9
_B = 16
_L = 4096
_L_OUT = _L - _K + 1  # 4089
_N_CORES = 8
_B_PER_CORE = _B // _N_CORES  # 2
_NCHUNK = 8  # 512-column chunks per batch
_CHUNK = 512

# k-major patch permutation: new index p = k*16 + c  <->  old index c*8 + k
_PERM = np.array([(p % _C) * _K + (p // _C) for p in range(_D)])


def _apply_ry_layer(psi, angles):
    # psi [N, DIM] float64; matches reference._apply_ry_layer
    for q in range(_NQ):
        half = angles[q] * 0.5
        c, s = np.cos(half), np.sin(half)
        left = 2 ** q
        p = psi.reshape(-1, left, 2, _DIM // (2 ** (q + 1)))
        a, b = p[:, :, 0, :].copy(), p[:, :, 1, :].copy()
        psi = np.stack([c * a - s * b, s * a + c * b], axis=2).reshape(-1, _DIM)
    return psi


def _build_amat(entangle_matrix, theta):
    """Collapse the circuit to the k-major-permuted 128x128 quadratic form."""
    U = np.asarray(entangle_matrix, dtype=np.float64)
    th = np.asarray(theta, dtype=np.float64)
    psi = np.eye(_DIM, dtype=np.float64)
    for l in range(th.shape[0]):
        psi = _apply_ry_layer(psi, th[l])
        psi = psi @ U.T
    M = psi.T  # state map: s -> M s
    z = np.concatenate([np.ones(_DIM // 2), -np.ones(_DIM // 2)])
    Md = M[:, :_D]
    A = Md.T @ (z[:, None] * Md)
    A_km = A[np.ix_(_PERM, _PERM)]
    return np.ascontiguousarray(A_km, dtype=np.float32)


_NC_CACHE = {}


def _build_nc():
    import concourse.tile as tile
    from concourse import bacc, mybir

    F32 = mybir.dt.float32
    F32R = mybir.dt.float32r
    AF = mybir.ActivationFunctionType

    nc = bacc.Bacc(
        "TRN2",
        target_bir_lowering=False,
        debug=False,
        num_devices=_N_CORES,
    )
    ngl = _B_PER_CORE * _NCHUNK  # 16 global chunks
    # flat x + 8 pad elements so the im2col window never reads out of bounds
    x = nc.dram_tensor(
        "x", [_B_PER_CORE * _C * _L + _K], F32, kind="ExternalInput"
    ).ap()
    # consts = [A_km | T2] with T2 [128, 64]: single ones-column at col 32.
    # A 32-wide window T2[:, 32-m : 64-m] is a selector matrix whose matmul
    # sums all partitions into output partition m (ones at in-window col m).
    consts = nc.dram_tensor(
        "consts", [_D, _D + 96], F32, kind="ExternalInput"
    ).ap()
    out = nc.dram_tensor(
        "out", [_B_PER_CORE * _NCHUNK, _CHUNK], F32, kind="ExternalOutput"
    ).ap()

    with tile.TileContext(nc) as tc:
        from contextlib import ExitStack

        with ExitStack() as ctx:
            const_pool = ctx.enter_context(tc.tile_pool(name="const", bufs=1))
            v_pool = ctx.enter_context(tc.tile_pool(name="v", bufs=2))
            p_pool = ctx.enter_context(tc.tile_pool(name="p", bufs=2))
            y_pool = ctx.enter_context(tc.tile_pool(name="y", bufs=2, space="PSUM"))
            red_pool = ctx.enter_context(tc.tile_pool(name="red", bufs=1, space="PSUM"))
            o_pool = ctx.enter_context(tc.tile_pool(name="o", bufs=1))

            c_sb = const_pool.tile([_D, _D + 96], F32)
            nc.scalar.dma_start(c_sb[:].bitcast(F32R), consts[:].bitcast(F32R))
            a_sb = c_sb[:, :_D]
            t2 = c_sb[:, _D:]

            def sel_num(g):
                # ones at within-window col g -> output partition g (num)
                return t2[:, 48 - g : 96 - g].bitcast(F32R)

            def sel_den(g):
                # ones at col 32+g -> output partition 32+g (den; 32-aligned
                # so the epilogue's partition-offset reads are legal)
                return t2[:, 16 - g : 64 - g].bitcast(F32R)

            # num rows 0..15, den rows 32..47, one PSUM bank total
            red = red_pool.tile([48, _CHUNK], F32)

            from bass_rust import AP as RawAP

            # V free size is _L+1 so its partition pitch (4097) can't be
            # coalesced with the 4096-element column runs by the DMA AP
            # balancer (a flat run crossing SBUF partitions is invalid).
            _LV = _L + 1
            _Q = 1024  # quarter width: DMA piece + y-tile width
            vs = []
            for b in range(_B_PER_CORE):
                v = v_pool.tile([_D, _LV], F32, tag="v")
                vs.append(v)
                # im2col in 4 column-quarters, alternating the two HWDGE
                # rings (sync / scalar) so all 16 SDMA engines run.
                # dst partition (k*16+c), col n <- x[b, c, n+k]; cols >=
                # L_OUT pick up neighboring-channel garbage (host discards).
                for q in range(4):
                    dst = v[:, q * _Q : (q + 1) * _Q].bitcast(F32R)
                    srcap = RawAP(
                        tensor=x.tensor, offset=b * _C * _L + q * _Q,
                        ap=[[1, _K], [_L, _C], [1, _Q]],
                    ).bitcast(F32R)
                    eng = nc.sync if q % 2 == 0 else nc.scalar
                    eng.dma_start(dst, srcap)

            mm_i = 0  # running index over all 64 reduction matmuls
            for b in range(_B_PER_CORE):
                v = vs[b]
                for h in range(2):  # 2048-wide halves for the squares
                    p2 = p_pool.tile([_D, 2 * _Q], F32, tag="p2")
                    nc.scalar.activation(
                        p2[:].bitcast(F32R),
                        v[:, h * 2 * _Q : (h + 1) * 2 * _Q],
                        AF.Square,
                    )
                    for qq in range(2):  # 1024-wide y tiles
                        base = h * 2 * _Q + qq * _Q
                        g0 = b * _NCHUNK + (base // _CHUNK)
                        y = y_pool.tile([_D, _Q], F32)
                        for s in range(2):
                            nc.tensor.matmul(
                                y[:, s * _CHUNK : (s + 1) * _CHUNK],
                                a_sb.bitcast(F32R),
                                v[:, base + s * _CHUNK : base + (s + 1) * _CHUNK]
                                .bitcast(F32R),
                                start=True, stop=True,
                            )
                        p1 = p_pool.tile([_D, _Q], F32, tag="p1")
                        nc.vector.tensor_mul(
                            p1[:].bitcast(F32R), v[:, base : base + _Q], y[:]
                        )
                        for s in range(2):
                            g = g0 + s
                            sl = slice(s * _CHUNK, (s + 1) * _CHUNK)
                            nc.tensor.matmul(
                                red[:], sel_num(g), p1[:, sl].bitcast(F32R),
                                start=(mm_i == 0), stop=(mm_i == 63),
                                skip_group_check=True,
                            )
                            mm_i += 1
                            sl2 = slice(qq * _Q + s * _CHUNK,
                                        qq * _Q + (s + 1) * _CHUNK)
                            nc.tensor.matmul(
                                red[:], sel_den(g), p2[:, sl2].bitcast(F32R),
                                start=(mm_i == 0), stop=(mm_i == 63),
                                skip_group_check=True,
                            )
                            mm_i += 1

            den_sb = o_pool.tile([16, _CHUNK], F32, tag="den")
            nc.scalar.activation(den_sb[:], red[32:48, :], AF.Copy, bias=1e-12)
            rden = o_pool.tile([16, _CHUNK], F32, tag="rden")
            nc.vector.reciprocal_approx_fast(rden[:], den_sb[:])
            out_sb = o_pool.tile([16, _CHUNK], F32, tag="outsb")
            nc.vector.tensor_mul(out_sb[:], red[0:16, :], rden[:])
            nc.sync.dma_start(out[:], out_sb[:])

    nc.compile()
    return nc


def get_nc():
    if "nc" not in _NC_CACHE:
        _NC_CACHE["nc"] = _build_nc()
    return _NC_CACHE["nc"]


def kernel(x, entangle_matrix, theta, _trace=False, **trace_kwargs):
    from concourse.bass_utils import run_bass_kernel_spmd

    x = np.asarray(x, dtype=np.float32)
    amat = _build_amat(entangle_matrix, theta)
    # T2: single ones-column at col 32; sliding 32-wide windows of T2 give
    # every selector matrix (ones exactly at within-block column g).
    t2 = np.zeros((_D, 96), dtype=np.float32)
    t2[:, 48] = 1.0
    consts = np.ascontiguousarray(np.concatenate([amat, t2], axis=1))

    nc = get_nc()
    pad = np.zeros(_K, dtype=np.float32)
    in_maps = [
        {
            "x": np.concatenate(
                [x[i * _B_PER_CORE : (i + 1) * _B_PER_CORE].reshape(-1), pad]
            ),
            "consts": consts,
        }
        for i in range(_N_CORES)
    ]
    res = run_bass_kernel_spmd(
        nc, in_maps, list(range(_N_CORES)), trace=_trace, **trace_kwargs
    )
    outs = []
    for i in range(_N_CORES):
        o = np.asarray(res.results[i]["out"], dtype=np.float32)
        outs.append(o.reshape(_B_PER_CORE, _NCHUNK * _CHUNK)[:, :_L_OUT])
    full = np.concatenate(outs, axis=0).reshape(_B, 1, 1, _L_OUT)
    if _trace:
        kernel._last_results = res
    return full
